# revision 8
# baseline (speedup 1.0000x reference)
"""Cross-attention Trainium2 kernel (8-core data-parallel over batch).

Per-core computation (one batch element per NeuronCore):
  q = x @ Wq; k = ctx @ Wk; v = ctx @ Wv
  attn = softmax((q k^T) / sqrt(dh)); out = attn @ v; y = out @ Wo + bo

Everything on-chip is kept "transposed" (feature dim on partitions, tokens on
the free dim) so every matmul streams 512-wide moving operands:
  xT   [qd, tok]    via DMA-XBAR transposes of bf16 x tiles (no PE cost)
  qT   [inner, tok] = Wq_chunk^T @ xT
  sT   [ctx, tok]   = k_hT^T @ q_hT
  e    [ctx, tok]   = exp(sT / 8)            (ACT; |scores/8| <~ 6)
  r    [pair, tok]  = half-ones selector matmuls (rowsums pre-broadcast
                      across 64 partitions, two heads per psum bank)
  outT [dh, tok]    = v_h^T @ e  then * (1/r) on DVE
  y    [tok, qd]    = outT^T @ Wo + bo

Dtypes: the PE requires both matmul operands 16-bit or both the exact same
32-bit type. x/ctx/Wq/Wk/Wv are cast-loaded to bf16 (SWDGE is the only DMA
path that casts, and the XBAR transpose needs 16-bit), so the q/k/v
projections run bf16 x bf16. Everything downstream (qT, kT, exp, v, outT,
and Wo loaded as raw fp32 on the HWDGE queue) stays fp32 and feeds the PE as
float32r x float32r: fp32r moving operands with free dim >= 256 stream at
1 cycle/row (bf16 speed) with ~fp32 accuracy, so scores/rowsum/av/final lose
no precision and no speed.

PE emission order per iteration (software-pipelined one group):
  qproj(g) | rowsum(g-1) | av(g-1) | scores(g) | final(g-1)
so every PE instruction's ACT/DVE-produced inputs are at least one PE block
old when the in-order PE queue reaches it.
"""

import numpy as np

import concourse.bass as bass
import concourse.tile as tile
from concourse import bacc, mybir
from concourse.bass_utils import run_bass_kernel_spmd
from concourse.masks import make_identity

F32 = mybir.dt.float32
F32R = mybir.dt.float32r
BF16 = mybir.dt.bfloat16

B, N, M = 8, 4096, 77
QD, CD, H, DH = 512, 768, 8, 64
INNER = H * DH  # 512
P = 128
S = 512  # token group size
NQC = QD // P  # 4
NCC = CD // P  # 6
NIC = INNER // P  # 4
NTS = S // P  # 4
SCALE = DH ** -0.5
MP = 128  # context length padded to full partition width

USE_XBAR = True   # x/ctx transposes on the DMA XBAR instead of the PE
W_F32R = True     # weights stay fp32 (HWDGE load), matmuls use float32r
SM_F32 = True     # softmax path (qT/kT/exp/v/outT) in fp32/f32r


def build_kernel(groups: int = N // S):
    nc = bacc.Bacc(None, target_bir_lowering=False, debug=False)

    x_d = nc.dram_tensor("x", [N, QD], F32, kind="ExternalInput")
    ctx_d = nc.dram_tensor("context", [M, CD], F32, kind="ExternalInput")
    wq_d = nc.dram_tensor("Wq", [QD, INNER], F32, kind="ExternalInput")
    wk_d = nc.dram_tensor("Wk", [CD, INNER], F32, kind="ExternalInput")
    wv_d = nc.dram_tensor("Wv", [CD, INNER], F32, kind="ExternalInput")
    wo_d = nc.dram_tensor("Wo", [INNER, QD], F32, kind="ExternalInput")
    bo_d = nc.dram_tensor("bo", [QD], F32, kind="ExternalInput")
    y_d = nc.dram_tensor("y", [N, QD], F32, kind="ExternalOutput")

    # float32r is fp32-width but PE-rounded: tiles feeding f32r matmuls are
    # declared f32r so their producers (ACT/DVE) round at write time, as the
    # BIR verifier requires.
    WDT = F32R if (W_F32R and SM_F32) else BF16  # Wo pairs with outT in final
    SDT = F32R if SM_F32 else BF16  # qT/kT/outT: f32r matmuls, dst base 0
    # the ISA forbids 32-bit matmuls writing at a dst partition offset, which
    # the pair-packed attention-output matmul needs (dst base side*64) -- so
    # the exp/v/sel operands of rowsum+av stay bf16.
    ADT = BF16

    def wcast(ap):
        return ap

    from contextlib import ExitStack

    with tile.TileContext(nc) as tc, ExitStack() as st:
        consts = st.enter_context(tc.tile_pool(name="consts", bufs=1))
        kvp = st.enter_context(tc.tile_pool(name="kv", bufs=1))
        xin = st.enter_context(tc.tile_pool(name="xin", bufs=3))
        xtp = st.enter_context(tc.tile_pool(name="xt", bufs=2))
        qtp = st.enter_context(tc.tile_pool(name="qt", bufs=2))
        expp = st.enter_context(tc.tile_pool(name="expp", bufs=2))
        rcp = st.enter_context(tc.tile_pool(name="rcp", bufs=2))
        outp = st.enter_context(tc.tile_pool(name="outp", bufs=2))
        yp = st.enter_context(tc.tile_pool(name="yp", bufs=2))

        # PSUM budget: 8 banks of [128, 512]xf32.
        ps_q = st.enter_context(tc.tile_pool(name="ps_q", bufs=2, space="PSUM"))
        ps_s = st.enter_context(tc.tile_pool(name="ps_s", bufs=2, space="PSUM"))
        ps_r = st.enter_context(tc.tile_pool(name="ps_r", bufs=2, space="PSUM"))
        ps_av = st.enter_context(tc.tile_pool(name="ps_av", bufs=2, space="PSUM"))

        # ---- loads --------------------------------------------------------
        # SWDGE (gpsimd) is the only casting DMA path and is serial: emit its
        # loads in first-use order (Wq and x0 gate the first qproj). Wo is
        # consumed as fp32r so it loads as raw fp32 on the ACT HWDGE queue,
        # in parallel with the SWDGE stream.
        def load_x(g):
            x_g = xin.tile([P, NTS, QD], BF16)
            nc.gpsimd.dma_start(
                out=x_g,
                in_=x_d[g * S : (g + 1) * S, :].rearrange("(t p) q -> p t q", p=P),
            )
            return x_g

        wq_sb = consts.tile([P, NQC, INNER], BF16)
        nc.gpsimd.dma_start(
            out=wq_sb, in_=wq_d.ap().rearrange("(c p) n -> p c n", p=P)
        )

        x_pre = [load_x(0)]

        ctx_sb = kvp.tile([MP, CD], BF16)
        nc.vector.memset(ctx_sb, 0.0)
        nc.gpsimd.dma_start(out=ctx_sb[:M, :], in_=ctx_d[:, :])

        wk_sb = consts.tile([P, NCC, INNER], BF16)
        nc.gpsimd.dma_start(
            out=wk_sb, in_=wk_d.ap().rearrange("(c p) n -> p c n", p=P)
        )
        wv_sb = consts.tile([P, NCC, INNER], BF16)
        nc.gpsimd.dma_start(
            out=wv_sb, in_=wv_d.ap().rearrange("(c p) n -> p c n", p=P)
        )

        x_pre.append(load_x(1))

        wo_sb = consts.tile([P, NIC, QD], WDT)
        if W_F32R:
            nc.scalar.dma_start(
                out=wo_sb,
                in_=wo_d.ap().rearrange("(c p) n -> p c n", p=P).bitcast(WDT),
            )
        else:
            nc.gpsimd.dma_start(
                out=wo_sb, in_=wo_d.ap().rearrange("(c p) n -> p c n", p=P)
            )
        bo_bc = consts.tile([P, QD], F32)
        bo_ap = bo_d.ap()
        nc.sync.dma_start(
            out=bo_bc, in_=bass.AP(bo_ap.tensor, bo_ap.offset, [[0, P], [1, QD]])
        )

        # half-ones selectors: a rowsum matmul with sel2[:, side] writes
        # sum_p(exp_h[p, t]) replicated across partitions side*64..side*64+63,
        # so the softmax denominator lands already broadcast, 2 heads per bank
        sel2_stage = consts.tile([M, 2, 2, DH], F32)
        nc.vector.memset(sel2_stage, 0.0)
        nc.vector.memset(sel2_stage[:, 0, 0, :], 1.0)
        nc.vector.memset(sel2_stage[:, 1, 1, :], 1.0)
        sel2 = consts.tile([M, 2, 2, DH], ADT)
        nc.vector.tensor_copy(out=sel2, in_=sel2_stage)

        if not USE_XBAR:
            identity = consts.tile([P, P], BF16)
            make_identity(nc, identity)

        # ---- context-side projections (tiny, one-time) --------------------
        ctxT = kvp.tile([P, NCC, MP], BF16)
        if USE_XBAR:
            nc.sync.dma_start(out=ctxT, in_=ctx_sb[:, :], transpose=True)
        else:
            for cc in range(NCC):
                pt = ps_s.tile([P, MP], BF16, tag="ps")
                nc.tensor.transpose(pt, ctx_sb[:, cc * P : (cc + 1) * P], identity)
                nc.vector.tensor_copy(out=ctxT[:, cc, :], in_=pt)

        kT = kvp.tile([P, NIC, MP], SDT)
        for ic in range(NIC):
            pk = ps_s.tile([P, S], F32, tag="ps")
            for cc in range(NCC):
                nc.tensor.matmul(
                    pk[:, :MP],
                    wcast(wk_sb[:, cc, ic * P : (ic + 1) * P]),
                    ctxT[:, cc, :],
                    start=(cc == 0),
                    stop=(cc == NCC - 1),
                )
            nc.vector.tensor_copy(out=kT[:, ic, :], in_=pk[:, :MP])

        v_sb = kvp.tile([MP, INNER], ADT)
        pv = ps_q.tile([MP, INNER], F32, tag="pq")
        for cc in range(NCC):
            nc.tensor.matmul(
                pv,
                ctxT[:, cc, :],
                wcast(wv_sb[:, cc, :]),
                start=(cc == 0),
                stop=(cc == NCC - 1),
            )
        nc.vector.tensor_copy(out=v_sb, in_=pv)

        # transpose x group g (one XBAR instruction covers all 16 128x128
        # blocks): xT[p, ts, c, j] = x_g[j, ts, c*128+p], token = ts*128+j
        def emit_xT(g):
            x_g = x_pre[g]
            xT = xtp.tile([P, NTS, NQC, P], BF16)
            if USE_XBAR:
                nc.sync.dma_start(out=xT, in_=x_g[:, :, :], transpose=True)
            else:
                for c in range(NQC):
                    pt = ps_s.tile([P, S], BF16, tag="ps")
                    for ts in range(NTS):
                        nc.tensor.transpose(
                            pt[:, ts * P : (ts + 1) * P],
                            x_g[:, ts, c * P : (c + 1) * P],
                            identity,
                        )
                    nc.vector.tensor_copy(out=xT[:, :, c, :], in_=pt)
            return xT

        xT_pre = [emit_xT(0)]

        # ---- software-pipelined main loop ---------------------------------
        state = {}  # g -> (exp_g,)

        def emit_qproj(g):
            xT = xT_pre[g]
            qT = qtp.tile([P, NIC, S], SDT)
            for ic in range(NIC):
                pq = ps_q.tile([P, S], F32, tag="pq")
                for c in range(NQC):
                    nc.tensor.matmul(
                        pq,
                        wcast(wq_sb[:, c, ic * P : (ic + 1) * P]),
                        xT[:, :, c, :],
                        start=(c == 0),
                        stop=(c == NQC - 1),
                    )
                nc.scalar.copy(out=qT[:, ic, :], in_=pq)
            return qT

        def emit_scores(g, qT):
            exp_g = expp.tile([MP, H, S], ADT)
            for pp in range(H // 2):
                for side in range(2):
                    par = side * DH
                    ps_sc = ps_s.tile([MP, S], F32, tag="ps")
                    nc.tensor.matmul(
                        ps_sc,
                        wcast(kT[par : par + DH, pp, :]),
                        wcast(qT[par : par + DH, pp, :]),
                        start=True,
                        stop=True,
                    )
                    nc.scalar.activation(
                        out=exp_g[:, 2 * pp + side, :],
                        in_=ps_sc,
                        func=mybir.ActivationFunctionType.Exp,
                        scale=SCALE,
                    )
            return exp_g

        def emit_rowsum(g, exp_g):
            rec_g = rcp.tile([P, H // 2, S], F32)
            for pp in range(H // 2):
                pr = ps_r.tile([P, S], F32, tag="pr")
                for side in range(2):
                    nc.tensor.matmul(
                        pr,
                        wcast(sel2[:, side]),
                        wcast(exp_g[:M, 2 * pp + side, :]),
                        start=(side == 0),
                        stop=(side == 1),
                    )
                nc.vector.reciprocal_approx_fast(out=rec_g[:, pp, :], in_=pr)
            return rec_g

        def emit_av(g, exp_g, rec_g):
            outT = outp.tile([P, NIC, S], SDT)
            for pp in range(H // 2):
                po = ps_av.tile([P, S], F32, tag="po")
                for side in range(2):
                    h = 2 * pp + side
                    nc.tensor.matmul(
                        po[side * DH : (side + 1) * DH, :],
                        wcast(v_sb[:, h * DH : (h + 1) * DH]),
                        wcast(exp_g[:, h, :]),
                        start=True,
                        stop=True,
                        tile_position=(0, side * DH),
                    )
                nc.vector.tensor_mul(
                    out=outT[:, pp, :], in0=po, in1=rec_g[:, pp, :]
                )
            return outT

        def emit_final(g, outT):
            tok = slice(g * S, (g + 1) * S)
            y_g = yp.tile([P, NTS, QD], F32)
            for ts in range(NTS):
                pf = ps_q.tile([P, QD], F32, tag="pq")
                for ic in range(NIC):
                    nc.tensor.matmul(
                        pf,
                        wcast(outT[:, ic, ts * P : (ts + 1) * P]),
                        wcast(wo_sb[:, ic, :]),
                        start=(ic == 0),
                        stop=(ic == NIC - 1),
                    )
                nc.vector.tensor_add(out=y_g[:, ts, :], in0=pf, in1=bo_bc)
            nc.sync.dma_start(
                out=y_d[tok, :].rearrange("(t p) q -> p t q", p=P), in_=y_g
            )

        pending = None  # (g, exp_g)
        for g in range(groups):
            if g + 2 < groups:
                x_pre.append(load_x(g + 2))
            qT = emit_qproj(g)
            if g + 1 < groups:
                xT_pre.append(emit_xT(g + 1))
            if pending is not None:
                pg, pexp = pending
                rec = emit_rowsum(pg, pexp)
                outT = emit_av(pg, pexp, rec)
            exp_g = emit_scores(g, qT)
            if pending is not None:
                emit_final(pg, outT)
            pending = (g, exp_g)

        pg, pexp = pending
        rec = emit_rowsum(pg, pexp)
        outT = emit_av(pg, pexp, rec)
        emit_final(pg, outT)

    nc.compile()
    return nc


_CACHE = {}


def _get_nc():
    if "nc" not in _CACHE:
        _CACHE["nc"] = build_kernel()
    return _CACHE["nc"]


def run(inputs, trace=False, **kw):
    nc = _get_nc()
    in_maps = []
    for i in range(B):
        m = {
            "x": np.asarray(inputs["x"][i], dtype=np.float32),
            "context": np.asarray(inputs["context"][i], dtype=np.float32),
            "Wq": np.asarray(inputs["Wq"], dtype=np.float32),
            "Wk": np.asarray(inputs["Wk"], dtype=np.float32),
            "Wv": np.asarray(inputs["Wv"], dtype=np.float32),
            "Wo": np.asarray(inputs["Wo"], dtype=np.float32),
            "bo": np.asarray(inputs["bo"], dtype=np.float32),
        }
        in_maps.append(m)
    res = run_bass_kernel_spmd(nc, in_maps, list(range(B)), trace=trace, **kw)
    out = np.stack([res.results[i]["y"] for i in range(B)], axis=0)
    return out, res


def kernel(**inputs):
    out, _ = run(inputs)
    return out
